# revision 4
# baseline (speedup 1.0000x reference)
"""Dense MoE layer on 8 NeuronCores, expert-parallel.

Math (per token t):
    gates = softmax(x @ Wg + bg)                      # [T, E]
    h_e   = gelu(x @ W1[e] + b1[e])                   # exact erf gelu
    y_e   = h_e @ W2[e] + b2[e]
    out   = sum_e gates[:, e] * y_e

Sharding: expert-parallel — core e computes g_e * y_e for its expert and
the host sums the 8 partial outputs.  E == n_cores == 8, so each core runs
two [4096,1024]x[1024,2048]-class matmuls (~34 GFLOP at fp32).

Device dataflow (per core, everything in "transposed" layout so both
matmuls consume natural weight layouts and no on-chip transposes happen):
    xT   [D, T]   (host-transposed input, replicated to all cores)
    hT   = Gelu(W1^T @ xT + b1)        via lhsT = W1 tiles  [d, h]
    yT   = (W2^T @ hT + b2) * g        via lhsT = W2 tiles  [h, d]
Per-expert gate without any cross-partition softmax:
    g_e[t] = 1 / sum_k exp((Wg_k - Wg_e) . x_t + (bg_k - bg_e))
The (Wg - Wg[:,e]) shift is precomputed on the host per core; on device:
8 accumulating matmuls -> dlogits [8, T], ACT Exp, then a ones[8,128]
matmul that simultaneously reduces over E and broadcasts the sum to 128
partitions, then one DVE reciprocal -> G [128, T].  The gate multiply is
fused into the PSUM->SBUF eviction of the second matmul
(scalar_tensor_tensor: (psum + b2) * G).

Matmuls use dt.float32r (fp32 operands truncated to FP22 in the PE):
1 cycle/row at N=512 (full rate, same as bf16) with ~2e-4 rel-l2 error
for these shapes (measured in numpy against fp64).
"""

import numpy as np

D, E, H = 1024, 8, 2048
B, S = 2, 2048
T = B * S            # 4096 tokens
TC = 512             # token chunk = matmul free dim = one PSUM bank (fp32)
NCH = T // TC        # 8 chunks
P = 128              # partitions
ND = D // P          # 8 d-tiles
NH = H // P          # 16 h-tiles

LAST_RESULTS = None   # BassKernelResults of the most recent run (for test.py)
_NC_CACHE = None


def _build():
    import concourse.bacc as bacc
    import concourse.bass as bass
    import concourse.mybir as mybir
    import concourse.tile as tile

    f32 = mybir.dt.float32
    f32r = mybir.dt.float32r
    AF = mybir.ActivationFunctionType
    OP = mybir.AluOpType
    PSUM = bass.MemorySpace.PSUM

    nc = bacc.Bacc(None)
    xT = nc.dram_tensor("xT", [D, T], f32r, kind="ExternalInput")
    w1 = nc.dram_tensor("w1", [D, H], f32r, kind="ExternalInput")
    w2 = nc.dram_tensor("w2", [H, D], f32r, kind="ExternalInput")
    wg = nc.dram_tensor("wg", [D, E], f32r, kind="ExternalInput")
    bg = nc.dram_tensor("bg", [E, 1], f32, kind="ExternalInput")
    b1 = nc.dram_tensor("b1", [P, NH], f32, kind="ExternalInput")
    b2 = nc.dram_tensor("b2", [P, ND], f32, kind="ExternalInput")
    on = nc.dram_tensor("ones", [E, P], f32r, kind="ExternalInput")
    yT = nc.dram_tensor("yT", [D, T], f32, kind="ExternalOutput")

    with tile.TileContext(nc) as tc:
        with (
            tc.tile_pool(name="wts", bufs=1) as wts,
            tc.tile_pool(name="xin", bufs=2) as xin,
            tc.tile_pool(name="hb", bufs=1) as hb,
            tc.tile_pool(name="yout", bufs=3) as yout,
            tc.tile_pool(name="gate", bufs=2) as gate,
            tc.tile_pool(name="php", bufs=3, space=PSUM) as php,
            tc.tile_pool(name="pyp", bufs=2, space=PSUM) as pyp,
            tc.tile_pool(name="pgp", bufs=1, space=PSUM) as pgp,
            tc.tile_pool(name="pSp", bufs=1, space=PSUM) as pSp,
        ):
            w1s = wts.tile([P, ND, H], f32r)
            w2s = wts.tile([P, NH, D], f32r)
            wgs = wts.tile([P, ND, E], f32r)
            b1s = wts.tile([P, NH], f32)
            b2s = wts.tile([P, ND], f32)
            bgs = wts.tile([E, 1], f32)
            ones = wts.tile([E, P], f32r)

            nc.sync.dma_start(wgs[:], wg.rearrange("(dt p) e -> p dt e", p=P))
            nc.sync.dma_start(bgs[:], bg[:])
            nc.sync.dma_start(b1s[:], b1[:])
            nc.sync.dma_start(b2s[:], b2[:])
            nc.sync.dma_start(ones[:], on[:])

            w1r = w1.rearrange("(dt p) h -> p dt h", p=P)
            for dt in range(ND):
                nc.sync.dma_start(w1s[:, dt, :], w1r[:, dt, :])
            w2r = w2.rearrange("(ht p) d -> p ht d", p=P)
            for ht in range(NH):
                nc.sync.dma_start(w2s[:, ht, :], w2r[:, ht, :])

            xTr = xT.rearrange("(dt p) t -> p dt t", p=P)
            for c in range(NCH):
                cs = slice(c * TC, (c + 1) * TC)
                xc = xin.tile([P, ND, TC], f32r, tag="xc")
                nc.sync.dma_start(xc[:], xTr[:, :, cs])

                # --- gate: G = 1 / sum_k exp(dlogits_k) broadcast to 128p ---
                pg = pgp.tile([E, TC], f32, tag="pg")
                for dt in range(ND):
                    nc.tensor.matmul(
                        pg[:],
                        wgs[:, dt, :],
                        xc[:, dt, :],
                        start=(dt == 0),
                        stop=(dt == ND - 1),
                    )
                ed = gate.tile([E, TC], f32r, tag="ed")
                nc.scalar.activation(ed[:], pg[:], AF.Exp, bias=bgs[:], scale=1.0)
                pS = pSp.tile([P, TC], f32, tag="pS")
                nc.tensor.matmul(
                    pS[:], ones[:], ed[:]
                )
                G = gate.tile([P, TC], f32, tag="G")
                nc.vector.reciprocal(G[:], pS[:])

                # --- hT = Gelu(W1^T @ xT + b1) ---
                hbuf = hb.tile([P, NH, TC], f32r, tag="hbuf")
                for ht in range(NH):
                    ph = php.tile([P, TC], f32, tag="ph")
                    for dt in range(ND):
                        nc.tensor.matmul(
                            ph[:],
                            w1s[:, dt, ht * P : (ht + 1) * P],
                            xc[:, dt, :],
                            start=(dt == 0),
                            stop=(dt == ND - 1),
                        )
                    nc.scalar.activation(
                        hbuf[:, ht, :], ph[:], AF.Gelu,
                        bias=b1s[:, ht : ht + 1], scale=1.0,
                    )

                # --- yT = (W2^T @ hT + b2) * G, evicted straight to DMA ---
                for dt in range(ND):
                    py = pyp.tile([P, TC], f32, tag="py")
                    for ht in range(NH):
                        nc.tensor.matmul(
                            py[:],
                            w2s[:, ht, dt * P : (dt + 1) * P],
                            hbuf[:, ht, :],
                            start=(ht == 0),
                            stop=(ht == NH - 1),
                        )
                    yt = yout.tile([P, TC], f32, tag="yt")
                    nc.vector.scalar_tensor_tensor(
                        yt[:], py[:], b2s[:, dt : dt + 1], G[:],
                        op0=OP.add, op1=OP.mult,
                    )
                    nc.sync.dma_start(yT[dt * P : (dt + 1) * P, cs], yt[:])

    nc.finalize()
    return nc


def kernel(x, Wg, bg, W1, b1, W2, b2):
    global LAST_RESULTS, _NC_CACHE
    from concourse.bass_utils import run_bass_kernel_spmd

    x = np.asarray(x, dtype=np.float32)
    Wg = np.asarray(Wg, dtype=np.float32)
    bg = np.asarray(bg, dtype=np.float32)
    W1 = np.asarray(W1, dtype=np.float32)
    b1 = np.asarray(b1, dtype=np.float32)
    W2 = np.asarray(W2, dtype=np.float32)
    b2 = np.asarray(b2, dtype=np.float32)

    xT = np.ascontiguousarray(x.reshape(T, D).T)          # [D, T]

    in_maps = []
    for e in range(E):
        wgp = np.ascontiguousarray(Wg - Wg[:, e : e + 1])  # [D, E]
        bgp = np.ascontiguousarray((bg - bg[e]).reshape(E, 1))
        in_maps.append(
            {
                "xT": xT,
                "w1": np.ascontiguousarray(W1[e]),
                "w2": np.ascontiguousarray(W2[e]),
                "wg": wgp,
                "bg": bgp,
                "b1": np.ascontiguousarray(b1[e].reshape(NH, P).T),
                "b2": np.ascontiguousarray(b2[e].reshape(ND, P).T),
                "ones": np.ones((E, P), dtype=np.float32),
            }
        )

    if _NC_CACHE is None:
        _NC_CACHE = _build()
    nc = _NC_CACHE

    res = run_bass_kernel_spmd(nc, in_maps, core_ids=list(range(E)))
    LAST_RESULTS = res

    acc = np.zeros((D, T), dtype=np.float64)
    for e in range(E):
        acc += res.results[e]["yT"]
    return np.ascontiguousarray(acc.T.astype(np.float32)).reshape(B, S, D)


# revision 15
# speedup vs baseline: 1.1614x; 1.1614x over previous
"""Dense MoE layer on 8 NeuronCores, expert-parallel.

Math (per token t):
    gates = softmax(x @ Wg + bg)                      # [T, E]
    h_e   = gelu(x @ W1[e] + b1[e])                   # exact erf gelu
    y_e   = h_e @ W2[e] + b2[e]
    out   = sum_e gates[:, e] * y_e

Sharding: expert-parallel — core e computes g_e * y_e for its expert and
the host sums the 8 partial outputs.  E == n_cores == 8, so each core runs
two [4096,1024]x[1024,2048]-class matmuls (~34 GFLOP at fp32).

Device dataflow (per core, everything in "transposed" layout so both
matmuls consume natural weight layouts and no on-chip transposes happen):
    xT   [D, T]   (host-transposed input, replicated to all cores)
    hT   = Gelu(W1^T @ xT + b1)        via lhsT = W1 tiles  [d, h]
    yT   = (W2^T @ hT + b2) * g        via lhsT = W2 tiles  [h, d]
Per-expert gate without any cross-partition softmax:
    g_e[t] = 1 / sum_k exp((Wg_k - Wg_e) . x_t + (bg_k - bg_e))
The (Wg - Wg[:,e]) shift is precomputed on the host per core; on device:
8 accumulating matmuls -> dlogits [8, T], ACT Exp, then a ones[8,128]
matmul that simultaneously reduces over E and broadcasts the sum to 128
partitions, then one DVE reciprocal -> G [128, T].  The gate multiply is
fused into the PSUM->SBUF eviction of the second matmul
(scalar_tensor_tensor: (psum + b2) * G).

Matmuls use dt.float32r (fp32 operands truncated to FP22 in the PE):
1 cycle/row at N=512 (full rate, same as bf16) with ~2e-4 rel-l2 error
for these shapes (measured in numpy against fp64).

DMA schedule: all input DMAs ride the single in-order qSp HWDGE queue in
exact consumption order (aux, xc0 per-d-tile, w1 per-h-tile, w2 per-
d-column-block, then xc1..7), so the PE starts ~6 us in instead of
waiting for the full 17 MB weight preload.  Output DMAs go through
gpsimd SWDGE so a not-yet-ready output can never block the input queue.
"""

import numpy as np

D, E, H = 1024, 8, 2048
B, S = 2, 2048
T = B * S            # 4096 tokens
TC = 512             # token chunk = matmul free dim = one PSUM bank (fp32)
NCH = T // TC        # 8 chunks
P = 128              # partitions
ND = D // P          # 8  d-tiles
NH = H // P          # 16 h-tiles

LAST_RESULTS = None   # BassKernelResults of the most recent run (for test.py)
_NC_CACHE = None

import os as _os
W_DT = _os.environ.get("MOE_W_DT", "f16")   # "f16" | "f32r"

# aux_f32 columns: [0:16] b1 (per h-tile), [16:24] b2 (per d-tile), [24] bg
AUXF_COLS = NH + ND + 1
# aux_f32r columns: [0:64] wg packed (dt-major, 8 cols each), [64:192] ones
AUXR_COLS = ND * E + P


def _build():
    import concourse.bacc as bacc
    import concourse.bass as bass
    import concourse.mybir as mybir
    import concourse.tile as tile

    f32 = mybir.dt.float32
    f32r = mybir.dt.float32r
    f16 = mybir.dt.float16
    wdt = {"f32r": f32r, "f16": f16}[W_DT]
    AF = mybir.ActivationFunctionType
    OP = mybir.AluOpType
    PSUM = bass.MemorySpace.PSUM

    nc = bacc.Bacc(None)
    xT = nc.dram_tensor("xT", [D, T], f32r, kind="ExternalInput")
    if W_DT == "f16":
        xT16 = nc.dram_tensor("xT16", [D, T], f16, kind="ExternalInput")
    w1 = nc.dram_tensor("w1", [D, H], wdt, kind="ExternalInput")
    w2 = nc.dram_tensor("w2", [H, D], wdt, kind="ExternalInput")
    auxf = nc.dram_tensor("auxf", [P, AUXF_COLS], f32, kind="ExternalInput")
    auxr = nc.dram_tensor("auxr", [P, AUXR_COLS], f32r, kind="ExternalInput")
    yT = nc.dram_tensor("yT", [D, T], f32, kind="ExternalOutput")

    with tile.TileContext(nc) as tc:
        with (
            tc.tile_pool(name="wts", bufs=1) as wts,
            tc.tile_pool(name="xin", bufs=2) as xin,
            tc.tile_pool(name="hb", bufs=1) as hb,
            tc.tile_pool(name="yout", bufs=3) as yout,
            tc.tile_pool(name="gate", bufs=2) as gate,
            tc.tile_pool(name="php", bufs=3, space=PSUM) as php,
            tc.tile_pool(name="pyp", bufs=3, space=PSUM) as pyp,
            tc.tile_pool(name="pgp", bufs=1, space=PSUM) as pgp,
            tc.tile_pool(name="pSp", bufs=1, space=PSUM) as pSp,
        ):
            w1s = wts.tile([P, NH, ND, P], wdt)     # w1s[p, ht, dt, hc]
            w2s = wts.tile([P, ND, NH, P], wdt)     # w2s[p, dt, ht, dc]
            axf = wts.tile([P, AUXF_COLS], f32)
            axr = wts.tile([P, AUXR_COLS], f32r)

            b1s = axf[:, 0:NH]
            b2s = axf[:, NH : NH + ND]
            bgs = axf[0:E, NH + ND : NH + ND + 1]
            ones = axr[0:E, ND * E : ND * E + P]

            nc.gpsimd.dma_start(axf[:], auxf[:])
            nc.gpsimd.dma_start(axr[:], auxr[:])

            # DRAM views for streaming weight loads in consumption order
            w1r = w1.rearrange("(dt p) (ht hc) -> p ht dt hc", p=P, hc=P)
            w2r = w2.rearrange("(ht p) (dt dc) -> p dt ht dc", p=P, dc=P)
            xTr = xT.rearrange("(dt p) t -> p dt t", p=P)
            xcs = [
                xin.tile([P, ND, TC], f32r, tag="xc", name=f"xc{c}")
                for c in range(NCH)
            ]
            if W_DT == "f16":
                xT16r = xT16.rearrange("(dt p) t -> p dt t", p=P)
                xc16s = [
                    xin.tile([P, ND, TC], f16, tag="xc16", name=f"xc16_{c}")
                    for c in range(NCH)
                ]
            else:
                xc16s = xcs

            # chunk-0 h-inputs first (smallest path to first matmul), then
            # w1 (first weight consumer), then gate input, then w2
            if W_DT == "f16":
                nc.sync.dma_start(xc16s[0][:], xT16r[:, :, 0:TC])
            for ht in range(NH):
                nc.sync.dma_start(w1s[:, ht], w1r[:, ht])
            nc.sync.dma_start(xcs[0][:], xTr[:, :, 0:TC])
            for dt in range(ND):
                nc.sync.dma_start(w2s[:, dt], w2r[:, dt])

            for c in range(NCH):
                cs = slice(c * TC, (c + 1) * TC)
                xc = xcs[c]
                xc16 = xc16s[c]
                if c > 0:
                    if W_DT == "f16":
                        nc.sync.dma_start(xc16[:], xT16r[:, :, cs])
                    nc.sync.dma_start(xc[:], xTr[:, :, cs])

                # --- hT = Gelu(W1^T @ xT + b1) ---
                hbuf = hb.tile([P, NH, TC], wdt, tag="hbuf")
                for ht in range(NH):
                    ph = php.tile([P, TC], f32, tag="ph")
                    for dt in range(ND):
                        nc.tensor.matmul(
                            ph[:],
                            w1s[:, ht, dt, :],
                            xc16[:, dt, :],
                            start=(dt == 0),
                            stop=(dt == ND - 1),
                        )
                    nc.scalar.activation(
                        hbuf[:, ht, :], ph[:], AF.Gelu,
                        bias=b1s[:, ht : ht + 1], scale=1.0,
                    )

                # --- gate: G = 1 / sum_k exp(dlogits_k), broadcast to 128p.
                # Emitted between the phases: G is first needed by the y
                # evictions, and keeping the PE's first chunk-0 work on the
                # (small, early) f16 path shortens the prologue. ---
                pg = pgp.tile([E, TC], f32, tag="pg")
                for dt in range(ND):
                    nc.tensor.matmul(
                        pg[:],
                        axr[:, dt * E : (dt + 1) * E],
                        xc[:, dt, :],
                        start=(dt == 0),
                        stop=(dt == ND - 1),
                    )
                ed = gate.tile([E, TC], f32r, tag="ed")
                nc.scalar.activation(ed[:], pg[:], AF.Exp, bias=bgs, scale=1.0)
                pS = pSp.tile([P, TC], f32, tag="pS")
                nc.tensor.matmul(pS[:], ones, ed[:])
                G = gate.tile([P, TC], f32, tag="G")
                nc.vector.reciprocal(G[:], pS[:])

                # --- yT = (W2^T @ hT + b2) * G, evicted straight to DMA ---
                for dt in range(ND):
                    py = pyp.tile([P, TC], f32, tag="py")
                    for ht in range(NH):
                        nc.tensor.matmul(
                            py[:],
                            w2s[:, dt, ht, :],
                            hbuf[:, ht, :],
                            start=(ht == 0),
                            stop=(ht == NH - 1),
                        )
                    yt = yout.tile([P, TC], f32, tag="yt")
                    nc.vector.scalar_tensor_tensor(
                        yt[:], py[:], b2s[:, dt : dt + 1], G[:],
                        op0=OP.add, op1=OP.mult,
                    )
                    out_eng = nc.sync if c == NCH - 1 else nc.gpsimd
                    out_eng.dma_start(yT[dt * P : (dt + 1) * P, cs], yt[:])

    nc.finalize()
    return nc


def kernel(x, Wg, bg, W1, b1, W2, b2):
    global LAST_RESULTS, _NC_CACHE
    from concourse.bass_utils import run_bass_kernel_spmd

    x = np.asarray(x, dtype=np.float32)
    Wg = np.asarray(Wg, dtype=np.float32)
    bg = np.asarray(bg, dtype=np.float32)
    W1 = np.asarray(W1, dtype=np.float32)
    b1 = np.asarray(b1, dtype=np.float32)
    W2 = np.asarray(W2, dtype=np.float32)
    b2 = np.asarray(b2, dtype=np.float32)

    xT = np.ascontiguousarray(x.reshape(T, D).T)          # [D, T]

    in_maps = []
    for e in range(E):
        wgp = Wg - Wg[:, e : e + 1]                        # [D, E]
        bgp = bg - bg[e]                                   # [E]

        auxf = np.zeros((P, AUXF_COLS), dtype=np.float32)
        auxf[:, 0:NH] = b1[e].reshape(NH, P).T
        auxf[:, NH : NH + ND] = b2[e].reshape(ND, P).T
        auxf[0:E, NH + ND] = bgp

        auxr = np.zeros((P, AUXR_COLS), dtype=np.float32)
        # wg packed: auxr[p, dt*E + k] = wgp[dt*P + p, k]
        auxr[:, 0 : ND * E] = (
            wgp.reshape(ND, P, E).transpose(1, 0, 2).reshape(P, ND * E)
        )
        auxr[0:E, ND * E : ND * E + P] = 1.0

        im = {
                "xT": xT,
                "w1": np.ascontiguousarray(
                    W1[e] if W_DT == "f32r" else W1[e].astype(np.float16)
                ),
                "w2": np.ascontiguousarray(
                    W2[e] if W_DT == "f32r" else W2[e].astype(np.float16)
                ),
                "auxf": auxf,
                "auxr": auxr,
        }
        if W_DT == "f16":
            im["xT16"] = xT.astype(np.float16)
        in_maps.append(im)

    if _NC_CACHE is None:
        _NC_CACHE = _build()
    nc = _NC_CACHE

    res = run_bass_kernel_spmd(nc, in_maps, core_ids=list(range(E)))
    LAST_RESULTS = res

    acc = np.zeros((D, T), dtype=np.float64)
    for e in range(E):
        acc += res.results[e]["yT"]
    return np.ascontiguousarray(acc.T.astype(np.float32)).reshape(B, S, D)
